# revision 46
# baseline (speedup 1.0000x reference)
"""Trainium2 Bass kernel for a dense transformer AttentionBlock.

Problem (fixed shapes): B=4, S=2048, D=512, H=8 heads (HD=64), FFN hidden 2048.
  qkv = x @ in_proj_w.T + b ; attn = softmax(q k^T / sqrt(64)) ; ctx = attn @ v
  x1 = LN(x + ctx @ out_w.T + out_b) ; out = LN(x1 + gelu(x1 @ w1.T + b1) @ w2.T + b2)

Sharding: 8 cores, zero collectives. Core c handles batch b=c//2, sequence half
h=c%2 (1024 query tokens). K/V are computed redundantly for the full 2048-token
sequence of the batch on both cores of a pair. One SPMD program for all cores:
for odd cores the host rolls x^T by -1024 columns so the core's own query
tokens always sit at columns [0,1024) (k-token order is irrelevant to softmax).

Schedule. The attention middle is ACT-bound on the 128 exp ops (~1.2us each);
everything else is arranged to hide under that stream:
  - Attention runs per (head-pair, 512-query half, 128-key tile): one
    [128,1024] score tile (both heads side by side), one 1024-wide exp, two
    ctx matmuls. ctx lags one slot behind its exp so the PE never waits on
    ACT; score tiles double-buffer in their own PSUM tag.
  - Input DMAs are ordered smallest-gate-first (wk/x/wq slivers for head-pair
    0 first, split HWDGE/SWDGE) so the first exp fires ~13us in. All other
    K/Q/V projections are uniform 4-matmul bursts with a dedicated 1-bank
    PSUM tag, woven one-per-key-tile into the exp stream.
  - Rowsums ride the ctx matmul via a per-head one-hot column block appended
    to V (eye+1e-30 so the whole [64:72] partition block stays finite and
    can be copied/inverted with aligned ops); each half's normalization is
    emitted into the next pass's stream.
  - PSUM is re-banked for the tail (attn_out triple-buffers 1-bank tiles so
    LayerNorm's ~2.5us PSUM hold never stalls the PE). The residual add is
    folded into each output matmul chain as an identity matmul; LN1
    mean/variance run on ACT accumulators because DVE is the tail
    bottleneck. FFN1 runs in 512-token halves so it starts as soon as the
    first four x1^T transposes land.
  - All matmul operands are bf16 (full-rate on PE, half DMA/SBUF); the
    residual/LayerNorm path stays fp32.
  - softmax without max-subtraction: scores are bounded (|s| <~ 1: 0.02-scale
    weights), so exp() is safe.
"""

import os
import numpy as np
import ml_dtypes
from contextlib import ExitStack

import concourse.bass as bass
import concourse.mybir as mybir
import concourse.tile as tile
from concourse import bacc
from concourse.bass_utils import run_bass_kernel_spmd

F32 = mybir.dt.float32
F32R = mybir.dt.float32r
BF16 = mybir.dt.bfloat16
AF = mybir.ActivationFunctionType
OP = mybir.AluOpType

B, S, D, H = 4, 2048, 512, 8
HD = D // H          # 64
F = 4 * D            # 2048
SQ = S // 2          # 1024 own query tokens per core
EPS = 1e-5
N_CORES = 8

# vext: per head 72 columns = [v(64) | 8 filler]; ones at col 72*h + 64 + h
VW = 72
VEXT_W = H * VW      # 576


def _emit(nc, flags):
    """Emit the whole per-core program. flags: dict of bools for optional ops.
    KERNEL_STOP_AFTER in {qkv, attn, ln1, ffn1} truncates for cost analysis."""
    stop_after = os.environ.get("KERNEL_STOP_AFTER", "")
    reps = int(os.environ.get("KERNEL_REPS", "1"))
    # ---- DRAM parameters ----
    xT_d = nc.declare_dram_parameter("xT", [D, S], BF16, isOutput=False)
    xown_d = nc.declare_dram_parameter("x_own", [SQ, D], F32, isOutput=False)
    wqkvT_d = nc.declare_dram_parameter("wqkvT", [D, 3 * D], BF16, isOutput=False)
    bqkv_d = nc.declare_dram_parameter("bqkv_pp", [128, 12], F32, isOutput=False)
    woutT_d = nc.declare_dram_parameter("woutT", [D, D], BF16, isOutput=False)
    w1T_d = nc.declare_dram_parameter("w1T", [D, F], BF16, isOutput=False)
    b1_d = nc.declare_dram_parameter("b1_pp", [128, 16], F32, isOutput=False)
    w2T_d = nc.declare_dram_parameter("w2T", [F, D], BF16, isOutput=False)
    assign_d = nc.declare_dram_parameter("assign", [8, 4, 128], F32R, isOutput=False)
    ident_d = nc.declare_dram_parameter("ident", [128, 128], F32R, isOutput=False)
    vecs_d = nc.declare_dram_parameter("vecs", [7, D], F32, isOutput=False)
    patt_d = nc.declare_dram_parameter("patt", [64], BF16, isOutput=False)
    out_d = nc.declare_dram_parameter("out", [SQ, D], F32, isOutput=True)

    VEC_ROW = {"bv": 0, "bout": 1, "b2": 2, "g1": 3, "bt1": 4, "g2": 5, "bt2": 6}

    dma = nc.gpsimd.dma_start
    hdma = nc.sync.dma_start

    def bcast(dst, src):
        # broadcast a 1-row DRAM source across 128 partitions
        src_b = bass.AP(tensor=src.tensor, offset=src.offset,
                        ap=[[0, 128]] + list(src.ap))
        dma(out=dst, in_=src_b)

    def bcast_row(pool, name, row):
        t = pool.tile([128, D], F32, tag=f"bc_{name}", name=f"bc_{name}")
        bcast(t[:], vecs_d[row])
        return t

    with tile.TileContext(nc) as tc:
      for _rep in range(reps):
        with ExitStack() as es:
            persist = es.enter_context(tc.tile_pool(name="persist", bufs=1))
            work = es.enter_context(tc.tile_pool(name="work", bufs=2))
            xo = es.enter_context(tc.tile_pool(name="xo", bufs=1))
            shr = es.enter_context(tc.tile_pool(name="shr", bufs=1))
            wf = es.enter_context(tc.tile_pool(name="wf", bufs=1))

            # PSUM is re-banked between the attention phase and the tail;
            # pspool[0] points at whichever PSUM pool is currently open
            pspool = [None]

            def ps_c(nm):
                # 2-bank ctx accumulator for one 512-query half (one buffer)
                return pspool[0].tile([128, 1024], F32, tag="c", name=nm, bufs=1)

            def ps_s(nm, dt=F32):
                # 2-bank double-buffered score psum
                return pspool[0].tile([128, 1024], dt, tag="s", name=nm, bufs=2)

            def ps_b(nm):
                # 1-bank double-buffered projection-burst psum: keeps the
                # interleaved K/Q/V bursts out of the score-tile rotation
                return pspool[0].tile([128, 512], F32, tag="b", name=nm, bufs=2)

            # ================= SBUF tiles =================
            # one combined tile per input stream (c indexes a free dim) so
            # each arrives in a single large DMA; K^T/Q^T are split per
            # 512-token quarter so scores gate on individual evacuations
            xq = [shr.tile([128, 4, 512], BF16, name=f"xq{t}") for t in range(4)]
            wq0 = shr.tile([128, 4, 128], BF16, name="wq0")
            wqR = shr.tile([128, 4, 384], BF16, name="wqR")
            wk0 = shr.tile([128, 4, 128], BF16, name="wk0")
            wkR = shr.tile([128, 4, 384], BF16, name="wkR")
            wv_sb = shr.tile([128, 4, 512], BF16, name="wv_sb")
            qTq = [[shr.tile([64 * 2, 512], BF16, name=f"qT{m}_{t}")
                    for t in range(2)] for m in range(4)]
            kTq = [[shr.tile([128, 512], BF16, name=f"kT{m}_{q}")
                    for q in range(4)] for m in range(4)]
            vx = [shr.tile([128, VEXT_W], BF16, name=f"vx{t}") for t in range(16)]
            rsum_sb = shr.tile([128, SQ], F32R, name="rsum_sb")
            actTc = [shr.tile([128, SQ], BF16, name=f"actT{c}") for c in range(4)]
            x2lo = shr.tile([128, 4, 512], BF16, name="x2lo")
            x2hi = shr.tile([128, 4, 512], BF16, name="x2hi")

            wqkv_r = wqkvT_d.ap().rearrange("(c p) m -> p c m", p=128)
            xT_r = xT_d.ap().rearrange("(c p) t -> p c t", p=128)
            # ---- critical DMAs, smallest-gate-first (the DMA stream is
            # serial: the first exp waits only on wk0+xq0+wq0 ~ 780KB) ----
            hdma(out=wk0[:], in_=wqkv_r[:, :, 512:640])
            hdma(out=xq[0][:], in_=xT_r[:, :, 0:512])
            hdma(out=wq0[:], in_=wqkv_r[:, :, 0:128])
            patt_sb = persist.tile([128, 64], BF16, name="patt_sb")
            bcast(patt_sb[:], patt_d[:])
            dma(out=wv_sb[:], in_=wqkv_r[:, :, 1024:1536])
            dma(out=xq[1][:], in_=xT_r[:, :, 512:1024])
            dma(out=wkR[:], in_=wqkv_r[:, :, 640:1024])
            dma(out=xq[2][:], in_=xT_r[:, :, 1024:1536])
            dma(out=xq[3][:], in_=xT_r[:, :, 1536:2048])
            dma(out=wqR[:], in_=wqkv_r[:, :, 128:512])
            # ---- small parameter DMAs (SWDGE queue, parallel) ----
            bqkv_sb = persist.tile([128, 12], F32, name="bqkv_sb")
            dma(out=bqkv_sb[:], in_=bqkv_d[:])
            b1_sb = persist.tile([128, 16], F32, name="b1_sb")
            dma(out=b1_sb[:], in_=b1_d[:])
            assign_sb = persist.tile([128, 4, 128], F32R, name="assign_sb")
            dma(out=assign_sb[64:72, :, :], in_=assign_d[:])
            ident_sb = persist.tile([128, 128], F32R, name="ident_sb")
            dma(out=ident_sb[:], in_=ident_d[:])
            eps_sb = persist.tile([128, 1], F32, name="eps_sb")
            nc.vector.memset(eps_sb[:], EPS)
            bc = {}
            for nm in ("bv", "bout", "b2", "g1", "bt1", "g2", "bt2"):
                if flags[nm]:
                    bc[nm] = bcast_row(persist, nm, VEC_ROW[nm])
            # ---- bulk DMAs (needed later; SWDGE queue) ----
            xown_sb = xo.tile([128, 8, D], F32R, name="xown_sb")
            dma(out=xown_sb[:],
                in_=xown_d.ap().rearrange("(j p) d -> p j d", p=128))
            woutT_sb = persist.tile([128, 4, D], BF16, name="woutT_sb")
            dma(out=woutT_sb[:],
                in_=woutT_d.ap().rearrange("(c p) m -> p c m", p=128))
            w1T_sb = wf.tile([128, 4, F], BF16, name="w1T_sb")
            for c in range(4):
                dma(out=w1T_sb[:, c, :], in_=w1T_d[128 * c:128 * c + 128, :])
            w2T_sb = wf.tile([128, 16, D], BF16, name="w2T_sb")
            for c in range(0, 16, 4):
                dma(out=w2T_sb[:, c:c + 4, :],
                    in_=w2T_d.ap().rearrange("(c p) m -> p c m",
                                             p=128)[:, c:c + 4, :])

            # ============ projection bursts (4 matmuls + 1 evac each) ======
            def k_group(mi, kh, tj):
                k_ps = ps_b(f"k_ps{mi}_{kh}_{tj}")
                wk_ap = (lambda c: wk0[:, c, :]) if mi == 0 else \
                    (lambda c: wkR[:, c, 128 * (mi - 1):128 * mi])
                for c in range(4):
                    nc.tensor.matmul(
                        k_ps[:, 0:512],
                        wk_ap(c),
                        xq[2 * kh + tj][:, c, :],
                        start=(c == 0), stop=(c == 3))
                dst = kTq[mi][2 * kh + tj][:]
                if flags["bqk"]:
                    nc.vector.tensor_scalar(dst, k_ps[:, 0:512],
                                            bqkv_sb[:, 4 + mi:5 + mi], None,
                                            OP.add)
                else:
                    nc.vector.tensor_copy(dst, k_ps[:, 0:512])

            def q_group(mi, tj):
                q_ps = ps_b(f"q_ps{mi}_{tj}")
                wq_ap = (lambda c: wq0[:, c, :]) if mi == 0 else \
                    (lambda c: wqR[:, c, 128 * (mi - 1):128 * mi])
                for c in range(4):
                    nc.tensor.matmul(
                        q_ps[:, 0:512],
                        wq_ap(c),
                        xq[tj][:, c, :],
                        start=(c == 0), stop=(c == 3))
                if flags["bqk"]:
                    nc.vector.tensor_scalar(qTq[mi][tj][:], q_ps[:, 0:512],
                                            bqkv_sb[:, mi:mi + 1], None, OP.add)
                else:
                    nc.vector.tensor_copy(qTq[mi][tj][:], q_ps[:, 0:512])

            def v_group(ti):
                v_ps = ps_b(f"v_ps{ti}")
                for c in range(4):
                    nc.tensor.matmul(
                        v_ps[:, 0:512],
                        xq[ti // 4][:, c, 128 * (ti % 4):128 * (ti % 4) + 128],
                        wv_sb[:, c, :],
                        start=(c == 0), stop=(c == 3))
                v_dst = vx[ti][:].rearrange("p (h e) -> p h e", e=VW)[:, :, 0:HD]
                v_src = v_ps[:, 0:512].rearrange("p (h e) -> p h e", e=HD)
                if flags["bv"]:
                    nc.vector.tensor_tensor(
                        v_dst, v_src,
                        bc["bv"][:].rearrange("p (h e) -> p h e", e=HD), OP.add)
                else:
                    nc.vector.tensor_copy(v_dst, v_src)
                nc.vector.tensor_copy(
                    vx[ti][:].rearrange("p (h e) -> p h e", e=VW)[:, :, HD:VW],
                    patt_sb[:].rearrange("p (h e) -> p h e", e=8))

            # bursts woven into the attention stream: (hp, qg) -> kt -> [fn].
            # Each V(ti) lands >=2 slots before ctx(0,0,ti) consumes it;
            # K(0,1,*) land before scores kt=8/12 of the first pass.
            def hp_bursts(hp, qg):
                if hp == 0 and qg == 0:
                    d = {t - 4: [lambda ti=t: v_group(ti)] for t in range(4, 8)}
                    d[4] = [lambda: k_group(0, 1, 0)]
                    d[5] = [lambda: k_group(0, 1, 1)]
                    d[14] = [lambda: q_group(0, 1)]
                    for t in range(8, 16):
                        d[t - 2] = d.get(t - 2, []) + [lambda ti=t: v_group(ti)]
                    return d
                if hp < 3 and qg == 1:
                    mi = hp + 1
                    d = {k + 2 * t: [lambda kh=k, tj=t: k_group(mi, kh, tj)]
                         for k in range(2) for t in range(2)}
                    d[4] = [lambda: q_group(mi, 0)]
                    d[5] = [lambda: q_group(mi, 1)]
                    return d
                return {}

            # ================= attention =================
            # ctx matmuls lag one kt slot behind their exp so the PE never
            # sits waiting on the activation engine
            with tc.tile_pool(name="ps1", bufs=2, space="PSUM") as _p1, \
                 tc.tile_pool(name="pp", bufs=4) as pp:
                pspool[0] = _p1
                # up-front, minimal gate for the first exp: K quarter-0 and
                # Q half-0 of head-pair 0, then the rest
                k_group(0, 0, 0)
                q_group(0, 0)
                for ti in range(4):
                    v_group(ti)
                k_group(0, 0, 1)
                if stop_after == "qkv":
                    return nc

                pend_norm = None
                for hp in range(4):
                  for qg in range(2):
                    bursts = hp_bursts(hp, qg)
                    c_ps = ps_c(f"c_ps{hp}_{qg}")
                    pend_ctx = None
                    for kt in range(16):
                        if kt == 7 and pend_norm is not None:
                            pend_norm()
                            pend_norm = None
                        s_ps = ps_s(f"s_ps{hp}_{qg}_{kt}")
                        for hh in range(2):
                            nc.tensor.matmul(
                                s_ps[:, 512 * hh:512 * hh + 512],
                                kTq[hp][kt // 4][
                                    64 * hh:64 * hh + 64,
                                    128 * (kt % 4):128 * (kt % 4) + 128],
                                qTq[hp][qg][64 * hh:64 * hh + 64, :],
                                start=True, stop=True)
                        p_sb = pp.tile([128, 1024], BF16, tag="p",
                                       name=f"p{hp}_{qg}_{kt}")
                        nc.scalar.activation(out=p_sb[:], in_=s_ps[:],
                                             func=AF.Exp)
                        if pend_ctx is not None:
                            pend_ctx()

                        def ctx(kt=kt, p_sb=p_sb):
                            for hh in range(2):
                                h = 2 * hp + hh
                                nc.tensor.matmul(
                                    c_ps[0:VW, 512 * hh:512 * hh + 512],
                                    vx[kt][:, VW * h:VW * h + VW],
                                    p_sb[:, 512 * hh:512 * hh + 512],
                                    start=(kt == 0), stop=(kt == 15))
                        pend_ctx = ctx
                        for fn in bursts.get(kt, []):
                            fn()
                    pend_ctx()
                    # evacuate this half's rowsums + ctx^T and invert; the
                    # normalize matmul+multiply are deferred into the next
                    # half's stream so the PE FIFO never waits on the DVE
                    # reciprocal chain
                    # all 8 rowsum rows are finite (the vext filler pattern is
                    # eye + 1e-30, so off-rows hold eps*rowsum): partition-
                    # aligned [64:72] block ops only. Rows of other head-pairs
                    # are overwritten, but their reciprocals were consumed by
                    # their (already-emitted) normalize step.
                    nc.vector.tensor_copy(
                        rsum_sb[64:72, 512 * qg:512 * qg + 512],
                        c_ps[64:72, 0:512])
                    nc.vector.tensor_tensor(
                        rsum_sb[64:72, 512 * qg:512 * qg + 512],
                        rsum_sb[64:72, 512 * qg:512 * qg + 512],
                        c_ps[64:72, 512:1024], OP.add)
                    for hh in range(2):
                        nc.vector.tensor_copy(
                            actTc[hp][64 * hh:64 * hh + 64,
                                      512 * qg:512 * qg + 512],
                            c_ps[0:64, 512 * hh:512 * hh + 512])
                    with nc.allow_low_precision(
                            reason="f32r holds fp32 bits; PE rounds on read"):
                        nc.vector.reciprocal(
                            rsum_sb[64:72, 512 * qg:512 * qg + 512],
                            rsum_sb[64:72, 512 * qg:512 * qg + 512])

                    def norm(hp=hp, qg=qg):
                        n_ps = ps_b(f"n_ps{hp}_{qg}")
                        nc.tensor.matmul(
                            n_ps[:, 0:512],
                            assign_sb[64:72, hp, :],
                            rsum_sb[64:72, 512 * qg:512 * qg + 512],
                            start=True, stop=True)
                        nc.vector.tensor_tensor(
                            actTc[hp][:, 512 * qg:512 * qg + 512],
                            actTc[hp][:, 512 * qg:512 * qg + 512],
                            n_ps[:, 0:512], OP.mult)
                    pend_norm = norm
                pend_norm()

            if stop_after == "attn":
                return nc

            # ---- attn_out (natural) + LN1; transposes after all j tiles ----
            # The residual is folded into the PSUM accumulation with one
            # identity matmul, so LayerNorm reads its input from PSUM.
            def layer_norm(j, acc_ps, out_ap, pre_b, g, bt):
                if pre_b is not None:
                    nc.vector.tensor_tensor(acc_ps, acc_ps, pre_b[:], OP.add)
                st = work.tile([128, 6], F32, tag="st", name=f"st{j}")
                nc.vector.bn_stats(out=st[:], in_=acc_ps)
                mv = work.tile([128, 2], F32, tag="mv", name=f"mv{j}")
                nc.vector.bn_aggr(out=mv[:], in_=st[:])
                sd = work.tile([128, 1], F32, tag="sd", name=f"sd{j}")
                nc.scalar.activation(out=sd[:], in_=mv[:, 1:2], func=AF.Sqrt,
                                     bias=eps_sb[:], scale=1.0)
                nc.vector.reciprocal(sd[:], sd[:])
                nc.vector.tensor_scalar(out_ap, acc_ps, mv[:, 0:1], sd[:],
                                        OP.subtract, OP.mult)
                if g is not None:
                    nc.vector.tensor_tensor(out_ap, out_ap, g[:], OP.mult)
                if bt is not None:
                    nc.vector.tensor_tensor(out_ap, out_ap, bt[:], OP.add)

            def layer_norm_act(j, acc_ps, out_ap, pre_b, g, bt):
                # mean/variance via ACT accumulators (DVE is the bottleneck
                # in the attn_out region; ACT is idle)
                if pre_b is not None:
                    nc.vector.tensor_tensor(acc_ps, acc_ps, pre_b[:], OP.add)
                zs = work.tile([128, D], F32, tag="zs", name=f"zs{j}")
                mu = work.tile([128, 1], F32, tag="mu", name=f"mu{j}")
                nc.scalar.activation(out=zs[:], in_=acc_ps, func=AF.Identity,
                                     scale=1.0 / D, accum_out=mu[:])
                z2 = work.tile([128, D], F32, tag="z2", name=f"z2{j}")
                ms = work.tile([128, 1], F32, tag="ms", name=f"ms{j}")
                nc.scalar.activation(out=z2[:], in_=acc_ps, func=AF.Square,
                                     scale=1.0 / np.sqrt(D), accum_out=ms[:])
                vv = work.tile([128, 1], F32, tag="vv", name=f"vv{j}")
                nc.vector.tensor_tensor(vv[:], mu[:], mu[:], OP.mult)
                nc.vector.tensor_tensor(vv[:], ms[:], vv[:], OP.subtract)
                sd = work.tile([128, 1], F32, tag="sd", name=f"sd{j}")
                nc.scalar.activation(out=sd[:], in_=vv[:], func=AF.Sqrt,
                                     bias=eps_sb[:], scale=1.0)
                nc.vector.reciprocal(sd[:], sd[:])
                nc.vector.tensor_scalar(out_ap, acc_ps, mu[:], sd[:],
                                        OP.subtract, OP.mult)
                if g is not None:
                    nc.vector.tensor_tensor(out_ap, out_ap, g[:], OP.mult)
                if bt is not None:
                    nc.vector.tensor_tensor(out_ap, out_ap, bt[:], OP.add)

            with tc.tile_pool(name="ps2", bufs=2, space="PSUM") as _p2, \
                 tc.tile_pool(name="hp_", bufs=1) as hpool:
                pspool[0] = _p2

                def ps_a(nm):
                    # 1-bank, triple-buffered: LayerNorm holds its input
                    # PSUM tile ~2.5us, so two slots are not enough
                    return _p2.tile([128, 512], F32, tag="a", name=nm, bufs=3)

                def ps_t(nm):
                    return _p2.tile([128, 512], F32R, tag="t", name=nm, bufs=1)

                def ps_f(nm):
                    return _p2.tile([128, 512], F32, tag="f", name=nm, bufs=4)

                def t_group(j):
                    # x2^T for token block j via PE transposes
                    t_ps = ps_t(f"t_ps{j}")
                    for i in range(4):
                        nc.tensor.transpose(t_ps[:, 128 * i:128 * i + 128],
                                            xown_sb[:, j, 128 * i:128 * i + 128],
                                            ident_sb[:])
                    x2 = x2lo if j < 4 else x2hi
                    nc.vector.tensor_copy(
                        x2[:, :, 128 * (j % 4):128 * (j % 4) + 128],
                        t_ps[:, 0:512].rearrange("p (i t) -> p i t", t=128))

                hT_sb = hpool.tile([128, 16, SQ], BF16, name="hT_sb")

                def f_group(m, tg):
                    # FFN1 for hidden block m, token half tg (gated only on
                    # that half's transposes)
                    f_ps = ps_f(f"f_ps{m}_{tg}")
                    x2 = x2lo if tg == 0 else x2hi
                    for c in range(4):
                        nc.tensor.matmul(
                            f_ps[:, 0:512],
                            w1T_sb[:, c, 128 * m:128 * m + 128],
                            x2[:, c, :],
                            start=(c == 0), stop=(c == 3))
                    nc.scalar.activation(
                        out=hT_sb[:, m, 512 * tg:512 * tg + 512],
                        in_=f_ps[:, 0:512], func=AF.Gelu,
                        bias=b1_sb[:, m:m + 1], scale=1.0)

                for j in range(8):
                    a_ps = ps_a(f"a_ps{j}")
                    for c in range(4):
                        nc.tensor.matmul(a_ps[:, 0:512],
                                         actTc[c][:, 128 * j:128 * j + 128],
                                         woutT_sb[:, c, :],
                                         start=(c == 0), stop=False)
                    nc.tensor.matmul(a_ps[:, 0:512], ident_sb[:],
                                     xown_sb[:, j, :], start=False, stop=True)
                    layer_norm_act(j, a_ps[:, 0:512], xown_sb[:, j, :],
                                   bc.get("bout"), bc.get("g1"), bc.get("bt1"))
                    if j >= 4:
                        t_group(j - 4)
                if stop_after == "ln1":
                    return nc
                # FFN1 token-half 0 starts as soon as transposes 0-3 landed;
                # transposes 4-7 are woven into its matmul stream
                for m in range(16):
                    f_group(m, 0)
                    if m % 4 == 3:
                        t_group(4 + m // 4)
                for m in range(16):
                    f_group(m, 1)

                if stop_after == "ffn1":
                    return nc
                for j in range(8):
                    y_ps = ps_a(f"y_ps{j}")
                    for fc in range(16):
                        nc.tensor.matmul(y_ps[:, 0:512],
                                         hT_sb[:, fc, 128 * j:128 * j + 128],
                                         w2T_sb[:, fc, :],
                                         start=(fc == 0), stop=False)
                    nc.tensor.matmul(y_ps[:, 0:512], ident_sb[:],
                                     xown_sb[:, j, :], start=False, stop=True)
                    o_sb = work.tile([128, D], F32, tag="o", name=f"o{j}")
                    layer_norm(8 + j, y_ps[:, 0:512], o_sb[:],
                               bc.get("b2"), bc.get("g2"), bc.get("bt2"))
                    hdma(out=out_d[128 * j:128 * j + 128, :], in_=o_sb[:])
    return nc


_NC_CACHE = {}


def _get_nc(flags):
    key = (tuple(sorted(flags.items())),
           os.environ.get("KERNEL_STOP_AFTER", ""),
           os.environ.get("KERNEL_REPS", "1"))
    if key not in _NC_CACHE:
        nc = bacc.Bacc("TRN2", target_bir_lowering=False, debug=False)
        _emit(nc, flags)
        nc.compile()
        _NC_CACHE[key] = nc
    return _NC_CACHE[key]


LAST_RESULTS = None


def make_in_maps(x, in_proj_w, in_proj_b, out_w, out_b, ln1_g, ln1_b, ln2_g,
                 ln2_b, ff_w1, ff_b1, ff_w2, ff_b2):
    x = np.asarray(x, dtype=np.float32)
    scale = np.float32(1.0 / np.sqrt(HD))

    wqkvT = np.ascontiguousarray(np.asarray(in_proj_w, np.float32).T)  # (D, 3D)
    wqkvT[:, :D] *= scale
    wqkvT = wqkvT.astype(ml_dtypes.bfloat16)
    bqkv = np.asarray(in_proj_b, np.float32).copy()
    bqkv[:D] *= scale
    bqkv_pp = np.ascontiguousarray(bqkv.reshape(12, 128).T)
    woutT = np.ascontiguousarray(
        np.asarray(out_w, np.float32).T).astype(ml_dtypes.bfloat16)
    w1T = np.ascontiguousarray(
        np.asarray(ff_w1, np.float32).T).astype(ml_dtypes.bfloat16)
    b1_pp = np.ascontiguousarray(np.asarray(ff_b1, np.float32).reshape(16, 128).T)
    w2T = np.ascontiguousarray(np.asarray(ff_w2, np.float32).T).astype(
        ml_dtypes.bfloat16)

    assign = np.zeros((8, 4, 128), np.float32)
    for h in range(8):
        i = h // 2
        lo = 64 * (h % 2)
        assign[h, i, lo:lo + 64] = 1.0
    ident = np.eye(128, dtype=np.float32)
    # eye + eps: every rowsum row of the ctx matmul stays finite, so the
    # whole [64:72] partition block can be copied/inverted with aligned ops
    patt = (np.eye(8, dtype=np.float32) + 1e-30).reshape(64).astype(
        ml_dtypes.bfloat16)

    bv = bqkv[2 * D:3 * D]
    vecs = np.stack([
        bv,
        np.asarray(out_b, np.float32),
        np.asarray(ff_b2, np.float32),
        np.asarray(ln1_g, np.float32),
        np.asarray(ln1_b, np.float32),
        np.asarray(ln2_g, np.float32),
        np.asarray(ln2_b, np.float32),
    ]).astype(np.float32)

    flags = {
        "bv": bool(np.any(bv != 0)),
        "bqk": bool(np.any(bqkv[:2 * D] != 0)),
        "bout": bool(np.any(vecs[1] != 0)),
        "b2": bool(np.any(vecs[2] != 0)),
        "g1": bool(np.any(vecs[3] != 1)),
        "bt1": bool(np.any(vecs[4] != 0)),
        "g2": bool(np.any(vecs[5] != 1)),
        "bt2": bool(np.any(vecs[6] != 0)),
    }

    in_maps = []
    for c in range(N_CORES):
        b, hh = c // 2, c % 2
        xb = x[b]
        xT = np.ascontiguousarray(xb.T) if hh == 0 else \
            np.ascontiguousarray(np.roll(xb.T, -SQ, axis=1))
        in_maps.append({
            "xT": xT.astype(ml_dtypes.bfloat16),
            "x_own": np.ascontiguousarray(xb[SQ * hh:SQ * (hh + 1)]),
            "wqkvT": wqkvT, "bqkv_pp": bqkv_pp, "woutT": woutT,
            "w1T": w1T, "b1_pp": b1_pp, "w2T": w2T,
            "assign": assign, "ident": ident, "vecs": vecs,
            "patt": patt,
        })
    return in_maps, flags


def kernel(x, in_proj_w, in_proj_b, out_w, out_b, ln1_g, ln1_b, ln2_g, ln2_b,
           ff_w1, ff_b1, ff_w2, ff_b2):
    global LAST_RESULTS
    in_maps, flags = make_in_maps(
        x, in_proj_w, in_proj_b, out_w, out_b, ln1_g, ln1_b, ln2_g, ln2_b,
        ff_w1, ff_b1, ff_w2, ff_b2)
    nc = _get_nc(flags)
    res = run_bass_kernel_spmd(
        nc, in_maps, core_ids=list(range(N_CORES)),
        trace=bool(int(os.environ.get("BASS_KERNEL_TRACE", "0"))))
    LAST_RESULTS = res

    out = np.empty((B, S, D), np.float32)
    for c in range(N_CORES):
        b, hh = c // 2, c % 2
        out[b, SQ * hh:SQ * (hh + 1)] = res.results[c]["out"]
    return out


# revision 56
# speedup vs baseline: 1.3179x; 1.3179x over previous
"""Trainium2 Bass kernel for a dense transformer AttentionBlock.

Problem (fixed shapes): B=4, S=2048, D=512, H=8 heads (HD=64), FFN hidden 2048.
  qkv = x @ in_proj_w.T + b ; attn = softmax(q k^T / sqrt(64)) ; ctx = attn @ v
  x1 = LN(x + ctx @ out_w.T + out_b) ; out = LN(x1 + gelu(x1 @ w1.T + b1) @ w2.T + b2)

Sharding: 8 cores, zero collectives. Core c handles batch b=c//2, sequence half
h=c%2 (1024 query tokens). K/V are computed redundantly for the full 2048-token
sequence of the batch on both cores of a pair. One SPMD program for all cores:
for odd cores the host rolls x^T by -1024 columns so the core's own query
tokens always sit at columns [0,1024) (k-token order is irrelevant to softmax).

Schedule. The attention middle is ACT-bound on the 128 exp ops (~1.2us each);
everything else is arranged to hide under that stream:
  - Attention runs per (head-pair, 512-query half, 128-key tile): one
    [128,1024] score tile (both heads side by side), one 1024-wide exp, two
    ctx matmuls. ctx lags one slot behind its exp so the PE never waits on
    ACT; score tiles double-buffer in their own PSUM tag.
  - Input DMAs are ordered smallest-gate-first (wk/x/wq slivers for head-pair
    0 first, split HWDGE/SWDGE) so the first exp fires ~11us in. All other
    K/Q/V projections are uniform 4-matmul bursts with a dedicated 1-bank
    PSUM tag, woven into the exp stream just-in-time for their consumers.
  - Rowsums ride the ctx matmul via a per-head one-hot column block appended
    to V (eye+1e-30 so the whole [64:72] partition block stays finite and
    can be copied/inverted with aligned ops); each half's normalization is
    emitted into the next pass's stream.
  - PSUM is re-banked for the tail (attn_out triple-buffers 1-bank tiles so
    LayerNorm's ~2.5us PSUM hold never stalls the PE). The residual add is
    folded into each output matmul chain as an identity matmul; LN1
    mean/variance run on ACT accumulators because DVE is the tail
    bottleneck. FFN1 runs in 512-token halves so it starts as soon as the
    first four x1^T transposes land.
  - All matmul operands are bf16 (full-rate on PE, half DMA/SBUF); the
    residual/LayerNorm path stays fp32.
  - softmax without max-subtraction: scores are bounded (|s| <~ 1: 0.02-scale
    weights), so exp() is safe.
"""

import os
import numpy as np
import ml_dtypes
from contextlib import ExitStack

import concourse.bass as bass
import concourse.mybir as mybir
import concourse.tile as tile
from concourse import bacc
from concourse.bass_utils import run_bass_kernel_spmd

F32 = mybir.dt.float32
F32R = mybir.dt.float32r
BF16 = mybir.dt.bfloat16
AF = mybir.ActivationFunctionType
OP = mybir.AluOpType

B, S, D, H = 4, 2048, 512, 8
HD = D // H          # 64
F = 4 * D            # 2048
SQ = S // 2          # 1024 own query tokens per core
EPS = 1e-5
N_CORES = 8

# vext: per head 72 columns = [v(64) | 8 filler]; ones at col 72*h + 64 + h
VW = 72
VEXT_W = H * VW      # 576


def _emit(nc, flags):
    """Emit the whole per-core program. flags: dict of bools for optional ops.
    KERNEL_STOP_AFTER in {qkv, attn, ln1, ffn1} truncates for cost analysis."""
    stop_after = os.environ.get("KERNEL_STOP_AFTER", "")
    reps = int(os.environ.get("KERNEL_REPS", "1"))
    # ---- DRAM parameters ----
    xT_d = nc.declare_dram_parameter("xT", [D, S], BF16, isOutput=False)
    xown_d = nc.declare_dram_parameter("x_own", [SQ, D], F32, isOutput=False)
    wqkvT_d = nc.declare_dram_parameter("wqkvT", [D, 3 * D], BF16, isOutput=False)
    bqkv_d = nc.declare_dram_parameter("bqkv_pp", [128, 12], F32, isOutput=False)
    woutT_d = nc.declare_dram_parameter("woutT", [D, D], BF16, isOutput=False)
    w1T_d = nc.declare_dram_parameter("w1T", [D, F], BF16, isOutput=False)
    b1_d = nc.declare_dram_parameter("b1_pp", [128, 16], F32, isOutput=False)
    w2T_d = nc.declare_dram_parameter("w2T", [F, D], BF16, isOutput=False)
    assign_d = nc.declare_dram_parameter("assign", [8, 4, 128], F32R, isOutput=False)
    ident_d = nc.declare_dram_parameter("ident", [128, 128], F32R, isOutput=False)
    vecs_d = nc.declare_dram_parameter("vecs", [7, D], F32, isOutput=False)
    patt_d = nc.declare_dram_parameter("patt", [64], BF16, isOutput=False)
    out_d = nc.declare_dram_parameter("out", [SQ, D], F32, isOutput=True)

    VEC_ROW = {"bv": 0, "bout": 1, "b2": 2, "g1": 3, "bt1": 4, "g2": 5, "bt2": 6}

    dma = nc.gpsimd.dma_start
    hdma = nc.sync.dma_start

    def bcast(dst, src):
        # broadcast a 1-row DRAM source across 128 partitions
        src_b = bass.AP(tensor=src.tensor, offset=src.offset,
                        ap=[[0, 128]] + list(src.ap))
        dma(out=dst, in_=src_b)

    def bcast_row(pool, name, row):
        t = pool.tile([128, D], F32, tag=f"bc_{name}", name=f"bc_{name}")
        bcast(t[:], vecs_d[row])
        return t

    with tile.TileContext(nc) as tc:
      for _rep in range(reps):
        with ExitStack() as es:
            persist = es.enter_context(tc.tile_pool(name="persist", bufs=1))
            work = es.enter_context(tc.tile_pool(name="work", bufs=2))
            xo = es.enter_context(tc.tile_pool(name="xo", bufs=1))
            shr = es.enter_context(tc.tile_pool(name="shr", bufs=1))
            wf = es.enter_context(tc.tile_pool(name="wf", bufs=1))

            # PSUM is re-banked between the attention phase and the tail;
            # pspool[0] points at whichever PSUM pool is currently open
            pspool = [None]

            def ps_c(nm):
                # 2-bank ctx accumulator for one 512-query half (one buffer)
                return pspool[0].tile([128, 1024], F32, tag="c", name=nm, bufs=1)

            def ps_s(nm, dt=F32):
                # 2-bank triple-buffered score psum: deep enough that the PE
                # runs ahead and amortizes interleaved projection bursts
                return pspool[0].tile([128, 1024], dt, tag="s", name=nm, bufs=3)

            def ps_b(nm):
                # projection bursts share the score rotation (use [:, 0:512])
                return pspool[0].tile([128, 1024], F32, tag="s", name=nm, bufs=3)

            # ================= SBUF tiles =================
            # one combined tile per input stream (c indexes a free dim) so
            # each arrives in a single large DMA; K^T/Q^T are split per
            # 512-token quarter so scores gate on individual evacuations
            xq = [shr.tile([128, 4, 512], BF16, name=f"xq{t}") for t in range(4)]
            wq0 = shr.tile([128, 4, 128], BF16, name="wq0")
            wqR = shr.tile([128, 4, 384], BF16, name="wqR")
            wk0 = shr.tile([128, 4, 128], BF16, name="wk0")
            wkR = shr.tile([128, 4, 384], BF16, name="wkR")
            wv_sb = shr.tile([128, 4, 512], BF16, name="wv_sb")
            qTq = [[shr.tile([64 * 2, 512], BF16, name=f"qT{m}_{t}")
                    for t in range(2)] for m in range(4)]
            kTq = [[shr.tile([128, 512], BF16, name=f"kT{m}_{q}")
                    for q in range(4)] for m in range(4)]
            vx = [shr.tile([128, VEXT_W], BF16, name=f"vx{t}") for t in range(16)]
            rsum_sb = shr.tile([128, SQ], F32R, name="rsum_sb")
            actL = [shr.tile([128, 512], BF16, name=f"actL{c}") for c in range(4)]
            actH = [shr.tile([128, 512], BF16, name=f"actH{c}") for c in range(4)]
            x2lo = shr.tile([128, 4, 512], BF16, name="x2lo")
            x2hi = shr.tile([128, 4, 512], BF16, name="x2hi")

            wqkv_r = wqkvT_d.ap().rearrange("(c p) m -> p c m", p=128)
            xT_r = xT_d.ap().rearrange("(c p) t -> p c t", p=128)
            # ---- critical DMAs, smallest-gate-first (the DMA stream is
            # serial: the first exp waits only on wk0+xq0+wq0 ~ 780KB) ----
            hdma(out=wk0[:], in_=wqkv_r[:, :, 512:640])
            hdma(out=xq[0][:], in_=xT_r[:, :, 0:512])
            hdma(out=wq0[:], in_=wqkv_r[:, :, 0:128])
            patt_sb = persist.tile([128, 64], BF16, name="patt_sb")
            bcast(patt_sb[:], patt_d[:])
            dma(out=wv_sb[:], in_=wqkv_r[:, :, 1024:1536])
            dma(out=xq[1][:], in_=xT_r[:, :, 512:1024])
            dma(out=wkR[:], in_=wqkv_r[:, :, 640:1024])
            dma(out=xq[2][:], in_=xT_r[:, :, 1024:1536])
            dma(out=xq[3][:], in_=xT_r[:, :, 1536:2048])
            dma(out=wqR[:], in_=wqkv_r[:, :, 128:512])
            # ---- small parameter DMAs (SWDGE queue, parallel) ----
            bqkv_sb = persist.tile([128, 12], F32, name="bqkv_sb")
            dma(out=bqkv_sb[:], in_=bqkv_d[:])
            b1_sb = persist.tile([128, 16], F32, name="b1_sb")
            dma(out=b1_sb[:], in_=b1_d[:])
            assign_sb = persist.tile([128, 4, 128], F32R, name="assign_sb")
            dma(out=assign_sb[64:72, :, :], in_=assign_d[:])
            ident_sb = persist.tile([128, 128], F32R, name="ident_sb")
            dma(out=ident_sb[:], in_=ident_d[:])
            eps_sb = persist.tile([128, 1], F32, name="eps_sb")
            nc.vector.memset(eps_sb[:], EPS)
            bc = {}
            for nm in ("bv", "bout", "b2", "g1", "bt1", "g2", "bt2"):
                if flags[nm]:
                    bc[nm] = bcast_row(persist, nm, VEC_ROW[nm])
            # ---- bulk DMAs (needed later; SWDGE queue) ----
            xown_sb = xo.tile([128, 8, D], F32R, name="xown_sb")
            dma(out=xown_sb[:],
                in_=xown_d.ap().rearrange("(j p) d -> p j d", p=128))
            woutT_sb = persist.tile([128, 4, D], BF16, name="woutT_sb")
            dma(out=woutT_sb[:],
                in_=woutT_d.ap().rearrange("(c p) m -> p c m", p=128))
            w1T_sb = wf.tile([128, 4, F], BF16, name="w1T_sb")
            for c in range(4):
                dma(out=w1T_sb[:, c, :], in_=w1T_d[128 * c:128 * c + 128, :])
            w2T_sb = wf.tile([128, 16, D], BF16, name="w2T_sb")
            for c in range(0, 16, 4):
                dma(out=w2T_sb[:, c:c + 4, :],
                    in_=w2T_d.ap().rearrange("(c p) m -> p c m",
                                             p=128)[:, c:c + 4, :])

            # ============ projection bursts (4 matmuls + 1 evac each) ======
            def k_group(mi, kh, tj):
                k_ps = ps_b(f"k_ps{mi}_{kh}_{tj}")
                wk_ap = (lambda c: wk0[:, c, :]) if mi == 0 else \
                    (lambda c: wkR[:, c, 128 * (mi - 1):128 * mi])
                for c in range(4):
                    nc.tensor.matmul(
                        k_ps[:, 0:512],
                        wk_ap(c),
                        xq[2 * kh + tj][:, c, :],
                        start=(c == 0), stop=(c == 3))
                dst = kTq[mi][2 * kh + tj][:]
                if flags["bqk"]:
                    nc.vector.tensor_scalar(dst, k_ps[:, 0:512],
                                            bqkv_sb[:, 4 + mi:5 + mi], None,
                                            OP.add)
                else:
                    nc.vector.tensor_copy(dst, k_ps[:, 0:512])

            def q_group(mi, tj):
                q_ps = ps_b(f"q_ps{mi}_{tj}")
                wq_ap = (lambda c: wq0[:, c, :]) if mi == 0 else \
                    (lambda c: wqR[:, c, 128 * (mi - 1):128 * mi])
                for c in range(4):
                    nc.tensor.matmul(
                        q_ps[:, 0:512],
                        wq_ap(c),
                        xq[tj][:, c, :],
                        start=(c == 0), stop=(c == 3))
                if flags["bqk"]:
                    nc.vector.tensor_scalar(qTq[mi][tj][:], q_ps[:, 0:512],
                                            bqkv_sb[:, mi:mi + 1], None, OP.add)
                else:
                    nc.vector.tensor_copy(qTq[mi][tj][:], q_ps[:, 0:512])

            def v_group(ti):
                v_ps = ps_b(f"v_ps{ti}")
                for c in range(4):
                    nc.tensor.matmul(
                        v_ps[:, 0:512],
                        xq[ti // 4][:, c, 128 * (ti % 4):128 * (ti % 4) + 128],
                        wv_sb[:, c, :],
                        start=(c == 0), stop=(c == 3))
                v_dst = vx[ti][:].rearrange("p (h e) -> p h e", e=VW)[:, :, 0:HD]
                v_src = v_ps[:, 0:512].rearrange("p (h e) -> p h e", e=HD)
                if flags["bv"]:
                    nc.vector.tensor_tensor(
                        v_dst, v_src,
                        bc["bv"][:].rearrange("p (h e) -> p h e", e=HD), OP.add)
                else:
                    nc.vector.tensor_copy(v_dst, v_src)
                nc.vector.tensor_copy(
                    vx[ti][:].rearrange("p (h e) -> p h e", e=VW)[:, :, HD:VW],
                    patt_sb[:].rearrange("p (h e) -> p h e", e=8))

            # bursts woven into the attention stream: (hp, qg) -> kt -> [fn].
            # Each V(ti) lands >=2 slots before ctx(0,0,ti) consumes it;
            # K(0,1,*) land before scores kt=8/12 of the first pass.
            def hp_bursts(hp, qg):
                if hp == 0 and qg == 0:
                    # everything except K000/Q00 rides the exp stream; V(ti)
                    # lands at least one slot before ctx(0,0,ti) consumes it
                    d = {0: [lambda: v_group(0), lambda: v_group(1)],
                         1: [lambda: v_group(2), lambda: v_group(3)],
                         2: [lambda: v_group(4), lambda: k_group(0, 0, 1)],
                         3: [lambda: v_group(5), lambda: v_group(6)],
                         4: [lambda: k_group(0, 1, 0), lambda: v_group(7)],
                         5: [lambda: k_group(0, 1, 1), lambda: v_group(8)],
                         14: [lambda: q_group(0, 1)]}
                    for t in range(9, 16):
                        d[t - 3] = [lambda ti=t: v_group(ti)]
                    return d
                if hp < 3 and qg == 1:
                    # spread every 3rd slot: each burst's PE debt recovers
                    # before the next lands, so the exp cadence never slips
                    mi = hp + 1
                    d = {3 * (k + 2 * t): [lambda kh=k, tj=t:
                                           k_group(mi, kh, tj)]
                         for k in range(2) for t in range(2)}
                    d[12] = [lambda: q_group(mi, 0)]
                    d[15] = [lambda: q_group(mi, 1)]
                    return d
                return {}

            # ================= attention =================
            # ctx matmuls lag one kt slot behind their exp so the PE never
            # sits waiting on the activation engine
            with tc.tile_pool(name="ps1", bufs=2, space="PSUM") as _p1, \
                 tc.tile_pool(name="pp", bufs=4) as pp:
                pspool[0] = _p1
                # up-front, minimal gate for the first exp: K quarter-0 and
                # Q half-0 of head-pair 0; everything else is a burst
                k_group(0, 0, 0)
                q_group(0, 0)
                if stop_after == "qkv":
                    return nc

                pend_norm = None
                for hp in range(4):
                  for qg in range(2):
                    bursts = hp_bursts(hp, qg)
                    c_ps = ps_c(f"c_ps{hp}_{qg}")
                    pend_ctx = None
                    for kt in range(16):
                        if kt == 7 and pend_norm is not None:
                            pend_norm()
                            pend_norm = None
                        s_ps = ps_s(f"s_ps{hp}_{qg}_{kt}")
                        for hh in range(2):
                            nc.tensor.matmul(
                                s_ps[:, 512 * hh:512 * hh + 512],
                                kTq[hp][kt // 4][
                                    64 * hh:64 * hh + 64,
                                    128 * (kt % 4):128 * (kt % 4) + 128],
                                qTq[hp][qg][64 * hh:64 * hh + 64, :],
                                start=True, stop=True)
                        p_sb = pp.tile([128, 1024], BF16, tag="p",
                                       name=f"p{hp}_{qg}_{kt}")
                        nc.scalar.activation(out=p_sb[:], in_=s_ps[:],
                                             func=AF.Exp)
                        if pend_ctx is not None:
                            pend_ctx()

                        def ctx(kt=kt, p_sb=p_sb):
                            for hh in range(2):
                                h = 2 * hp + hh
                                nc.tensor.matmul(
                                    c_ps[0:VW, 512 * hh:512 * hh + 512],
                                    vx[kt][:, VW * h:VW * h + VW],
                                    p_sb[:, 512 * hh:512 * hh + 512],
                                    start=(kt == 0), stop=(kt == 15))
                        pend_ctx = ctx
                        for fn in bursts.get(kt, []):
                            fn()
                    pend_ctx()
                    # evacuate this half's rowsums + ctx^T and invert; the
                    # normalize matmul+multiply are deferred into the next
                    # half's stream so the PE FIFO never waits on the DVE
                    # reciprocal chain
                    # all 8 rowsum rows are finite (the vext filler pattern is
                    # eye + 1e-30, so off-rows hold eps*rowsum): partition-
                    # aligned [64:72] block ops only. Rows of other head-pairs
                    # are overwritten, but their reciprocals were consumed by
                    # their (already-emitted) normalize step.
                    nc.vector.tensor_copy(
                        rsum_sb[64:72, 512 * qg:512 * qg + 512],
                        c_ps[64:72, 0:512])
                    nc.vector.tensor_tensor(
                        rsum_sb[64:72, 512 * qg:512 * qg + 512],
                        rsum_sb[64:72, 512 * qg:512 * qg + 512],
                        c_ps[64:72, 512:1024], OP.add)
                    act_t = (actL if qg == 0 else actH)[hp]
                    for hh in range(2):
                        nc.vector.tensor_copy(
                            act_t[64 * hh:64 * hh + 64, :],
                            c_ps[0:64, 512 * hh:512 * hh + 512])
                    with nc.allow_low_precision(
                            reason="f32r holds fp32 bits; PE rounds on read"):
                        nc.vector.reciprocal(
                            rsum_sb[64:72, 512 * qg:512 * qg + 512],
                            rsum_sb[64:72, 512 * qg:512 * qg + 512])

                    def norm(hp=hp, qg=qg):
                        n_ps = ps_b(f"n_ps{hp}_{qg}")
                        nc.tensor.matmul(
                            n_ps[:, 0:512],
                            assign_sb[64:72, hp, :],
                            rsum_sb[64:72, 512 * qg:512 * qg + 512],
                            start=True, stop=True)
                        act_t = (actL if qg == 0 else actH)[hp]
                        nc.vector.tensor_tensor(
                            act_t[:], act_t[:], n_ps[:, 0:512], OP.mult)
                    pend_norm = norm
                pend_norm()

            # prefetch the sqrt ACT table set (the exp stream is done): the
            # ~2.7us load runs in the attention->tail transition window
            # instead of inside LayerNorm(0)'s critical chain
            warm_sb = work.tile([128, 1], F32, tag="warm", name="warm_sqrt")
            nc.scalar.activation(out=warm_sb[:], in_=eps_sb[:], func=AF.Sqrt,
                                 scale=1.0)

            if stop_after == "attn":
                return nc

            # ---- attn_out (natural) + LN1; transposes after all j tiles ----
            # The residual is folded into the PSUM accumulation with one
            # identity matmul, so LayerNorm reads its input from PSUM.
            def layer_norm(j, acc_ps, out_ap, pre_b, g, bt):
                if pre_b is not None:
                    nc.vector.tensor_tensor(acc_ps, acc_ps, pre_b[:], OP.add)
                st = work.tile([128, 6], F32, tag="st", name=f"st{j}")
                nc.vector.bn_stats(out=st[:], in_=acc_ps)
                mv = work.tile([128, 2], F32, tag="mv", name=f"mv{j}")
                nc.vector.bn_aggr(out=mv[:], in_=st[:])
                sd = work.tile([128, 1], F32, tag="sd", name=f"sd{j}")
                nc.scalar.activation(out=sd[:], in_=mv[:, 1:2], func=AF.Sqrt,
                                     bias=eps_sb[:], scale=1.0)
                nc.vector.reciprocal(sd[:], sd[:])
                nc.vector.tensor_scalar(out_ap, acc_ps, mv[:, 0:1], sd[:],
                                        OP.subtract, OP.mult)
                if g is not None:
                    nc.vector.tensor_tensor(out_ap, out_ap, g[:], OP.mult)
                if bt is not None:
                    nc.vector.tensor_tensor(out_ap, out_ap, bt[:], OP.add)

            def layer_norm_act(j, acc_ps, out_ap, pre_b, g, bt):
                # mean/variance via ACT accumulators (DVE is the bottleneck
                # in the attn_out region; ACT is idle)
                if pre_b is not None:
                    nc.vector.tensor_tensor(acc_ps, acc_ps, pre_b[:], OP.add)
                zs = work.tile([128, D], F32, tag="zs", name=f"zs{j}")
                mu = work.tile([128, 1], F32, tag="mu", name=f"mu{j}")
                nc.scalar.activation(out=zs[:], in_=acc_ps, func=AF.Identity,
                                     scale=1.0 / D, accum_out=mu[:])
                z2 = work.tile([128, D], F32, tag="z2", name=f"z2{j}")
                ms = work.tile([128, 1], F32, tag="ms", name=f"ms{j}")
                nc.scalar.activation(out=z2[:], in_=acc_ps, func=AF.Square,
                                     scale=1.0 / np.sqrt(D), accum_out=ms[:])
                vv = work.tile([128, 1], F32, tag="vv", name=f"vv{j}")
                nc.vector.tensor_tensor(vv[:], mu[:], mu[:], OP.mult)
                nc.vector.tensor_tensor(vv[:], ms[:], vv[:], OP.subtract)
                sd = work.tile([128, 1], F32, tag="sd", name=f"sd{j}")
                nc.scalar.activation(out=sd[:], in_=vv[:], func=AF.Sqrt,
                                     bias=eps_sb[:], scale=1.0)
                nc.vector.reciprocal(sd[:], sd[:])
                nc.vector.tensor_scalar(out_ap, acc_ps, mu[:], sd[:],
                                        OP.subtract, OP.mult)
                if g is not None:
                    nc.vector.tensor_tensor(out_ap, out_ap, g[:], OP.mult)
                if bt is not None:
                    nc.vector.tensor_tensor(out_ap, out_ap, bt[:], OP.add)

            with tc.tile_pool(name="ps2", bufs=2, space="PSUM") as _p2, \
                 tc.tile_pool(name="hp_", bufs=1) as hpool:
                pspool[0] = _p2

                def ps_a(nm):
                    # 1-bank, quad-buffered: LayerNorm holds its input PSUM
                    # tile ~3us, so fewer slots stall the attn_out stream
                    return _p2.tile([128, 512], F32, tag="a", name=nm, bufs=4)

                def ps_t(nm):
                    return _p2.tile([128, 512], F32R, tag="t", name=nm, bufs=1)

                def ps_f(nm):
                    return _p2.tile([128, 512], F32, tag="f", name=nm, bufs=3)

                def t_group(j):
                    # x2^T for token block j via PE transposes
                    t_ps = ps_t(f"t_ps{j}")
                    for i in range(4):
                        nc.tensor.transpose(t_ps[:, 128 * i:128 * i + 128],
                                            xown_sb[:, j, 128 * i:128 * i + 128],
                                            ident_sb[:])
                    x2 = x2lo if j < 4 else x2hi
                    nc.vector.tensor_copy(
                        x2[:, :, 128 * (j % 4):128 * (j % 4) + 128],
                        t_ps[:, 0:512].rearrange("p (i t) -> p i t", t=128))

                hT_sb = hpool.tile([128, 16, SQ], BF16, name="hT_sb")

                def f_group(m, tg):
                    # FFN1 for hidden block m, token half tg (gated only on
                    # that half's transposes)
                    f_ps = ps_f(f"f_ps{m}_{tg}")
                    x2 = x2lo if tg == 0 else x2hi
                    for c in range(4):
                        nc.tensor.matmul(
                            f_ps[:, 0:512],
                            w1T_sb[:, c, 128 * m:128 * m + 128],
                            x2[:, c, :],
                            start=(c == 0), stop=(c == 3))
                    nc.scalar.activation(
                        out=hT_sb[:, m, 512 * tg:512 * tg + 512],
                        in_=f_ps[:, 0:512], func=AF.Gelu,
                        bias=b1_sb[:, m:m + 1], scale=1.0)

                for j in range(8):
                    a_ps = ps_a(f"a_ps{j}")
                    act_t = actL if j < 4 else actH
                    for c in range(4):
                        nc.tensor.matmul(
                            a_ps[:, 0:512],
                            act_t[c][:, 128 * (j % 4):128 * (j % 4) + 128],
                            woutT_sb[:, c, :],
                            start=(c == 0), stop=False)
                    nc.tensor.matmul(a_ps[:, 0:512], ident_sb[:],
                                     xown_sb[:, j, :], start=False, stop=True)
                    layer_norm_act(j, a_ps[:, 0:512], xown_sb[:, j, :],
                                   bc.get("bout"), bc.get("g1"), bc.get("bt1"))
                    if j >= 4:
                        t_group(j - 4)
                if stop_after == "ln1":
                    return nc
                # FFN1 token-half 0 starts as soon as transposes 0-3 landed;
                # transposes 4-7 are woven into its matmul stream
                for m in range(16):
                    f_group(m, 0)
                    if m % 4 == 3:
                        t_group(4 + m // 4)
                for m in range(16):
                    f_group(m, 1)

                if stop_after == "ffn1":
                    return nc
                for j in range(8):
                    y_ps = ps_a(f"y_ps{j}")
                    for fc in range(16):
                        nc.tensor.matmul(y_ps[:, 0:512],
                                         hT_sb[:, fc, 128 * j:128 * j + 128],
                                         w2T_sb[:, fc, :],
                                         start=(fc == 0), stop=False)
                    nc.tensor.matmul(y_ps[:, 0:512], ident_sb[:],
                                     xown_sb[:, j, :], start=False, stop=True)
                    o_sb = work.tile([128, D], F32, tag="o", name=f"o{j}")
                    layer_norm(8 + j, y_ps[:, 0:512], o_sb[:],
                               bc.get("b2"), bc.get("g2"), bc.get("bt2"))
                    hdma(out=out_d[128 * j:128 * j + 128, :], in_=o_sb[:])
    return nc


_NC_CACHE = {}


def _get_nc(flags):
    key = (tuple(sorted(flags.items())),
           os.environ.get("KERNEL_STOP_AFTER", ""),
           os.environ.get("KERNEL_REPS", "1"))
    if key not in _NC_CACHE:
        nc = bacc.Bacc("TRN2", target_bir_lowering=False, debug=False)
        _emit(nc, flags)
        nc.compile()
        _NC_CACHE[key] = nc
    return _NC_CACHE[key]


LAST_RESULTS = None


def make_in_maps(x, in_proj_w, in_proj_b, out_w, out_b, ln1_g, ln1_b, ln2_g,
                 ln2_b, ff_w1, ff_b1, ff_w2, ff_b2):
    x = np.asarray(x, dtype=np.float32)
    scale = np.float32(1.0 / np.sqrt(HD))

    wqkvT = np.ascontiguousarray(np.asarray(in_proj_w, np.float32).T)  # (D, 3D)
    wqkvT[:, :D] *= scale
    wqkvT = wqkvT.astype(ml_dtypes.bfloat16)
    bqkv = np.asarray(in_proj_b, np.float32).copy()
    bqkv[:D] *= scale
    bqkv_pp = np.ascontiguousarray(bqkv.reshape(12, 128).T)
    woutT = np.ascontiguousarray(
        np.asarray(out_w, np.float32).T).astype(ml_dtypes.bfloat16)
    w1T = np.ascontiguousarray(
        np.asarray(ff_w1, np.float32).T).astype(ml_dtypes.bfloat16)
    b1_pp = np.ascontiguousarray(np.asarray(ff_b1, np.float32).reshape(16, 128).T)
    w2T = np.ascontiguousarray(np.asarray(ff_w2, np.float32).T).astype(
        ml_dtypes.bfloat16)

    assign = np.zeros((8, 4, 128), np.float32)
    for h in range(8):
        i = h // 2
        lo = 64 * (h % 2)
        assign[h, i, lo:lo + 64] = 1.0
    ident = np.eye(128, dtype=np.float32)
    # eye + eps: every rowsum row of the ctx matmul stays finite, so the
    # whole [64:72] partition block can be copied/inverted with aligned ops
    patt = (np.eye(8, dtype=np.float32) + 1e-30).reshape(64).astype(
        ml_dtypes.bfloat16)

    bv = bqkv[2 * D:3 * D]
    vecs = np.stack([
        bv,
        np.asarray(out_b, np.float32),
        np.asarray(ff_b2, np.float32),
        np.asarray(ln1_g, np.float32),
        np.asarray(ln1_b, np.float32),
        np.asarray(ln2_g, np.float32),
        np.asarray(ln2_b, np.float32),
    ]).astype(np.float32)

    flags = {
        "bv": bool(np.any(bv != 0)),
        "bqk": bool(np.any(bqkv[:2 * D] != 0)),
        "bout": bool(np.any(vecs[1] != 0)),
        "b2": bool(np.any(vecs[2] != 0)),
        "g1": bool(np.any(vecs[3] != 1)),
        "bt1": bool(np.any(vecs[4] != 0)),
        "g2": bool(np.any(vecs[5] != 1)),
        "bt2": bool(np.any(vecs[6] != 0)),
    }

    in_maps = []
    for c in range(N_CORES):
        b, hh = c // 2, c % 2
        xb = x[b]
        xT = np.ascontiguousarray(xb.T) if hh == 0 else \
            np.ascontiguousarray(np.roll(xb.T, -SQ, axis=1))
        in_maps.append({
            "xT": xT.astype(ml_dtypes.bfloat16),
            "x_own": np.ascontiguousarray(xb[SQ * hh:SQ * (hh + 1)]),
            "wqkvT": wqkvT, "bqkv_pp": bqkv_pp, "woutT": woutT,
            "w1T": w1T, "b1_pp": b1_pp, "w2T": w2T,
            "assign": assign, "ident": ident, "vecs": vecs,
            "patt": patt,
        })
    return in_maps, flags


def kernel(x, in_proj_w, in_proj_b, out_w, out_b, ln1_g, ln1_b, ln2_g, ln2_b,
           ff_w1, ff_b1, ff_w2, ff_b2):
    global LAST_RESULTS
    in_maps, flags = make_in_maps(
        x, in_proj_w, in_proj_b, out_w, out_b, ln1_g, ln1_b, ln2_g, ln2_b,
        ff_w1, ff_b1, ff_w2, ff_b2)
    nc = _get_nc(flags)
    res = run_bass_kernel_spmd(
        nc, in_maps, core_ids=list(range(N_CORES)),
        trace=bool(int(os.environ.get("BASS_KERNEL_TRACE", "0"))))
    LAST_RESULTS = res

    out = np.empty((B, S, D), np.float32)
    for c in range(N_CORES):
        b, hh = c // 2, c % 2
        out[b, SQ * hh:SQ * (hh + 1)] = res.results[c]["out"]
    return out
